# revision 6
# baseline (speedup 1.0000x reference)
"""Multi-head attention (B=2, S=4096, D=512, H=8) on 8 NeuronCores.

Sharding: data-parallel on batch x head-pair-parallel.  Core c handles
batch b = c//4 and heads (2*(c%4), 2*(c%4)+1).  Each core computes its
[4096, 128] slice of the output; the host scatters inputs / gathers
outputs.

Per-core kernel (Bass/Tile), fp16 operands (fp32 PSUM accumulate):
  - Projections pipelined with the first query chunk's attention: xT
    blocks DMA in; K^T / Q^T ([128, S], head dims on partitions, bias
    added on ACT) and V ([k, 80]-per-(ktile,head) tiles with a ones
    column, bias added on DVE) are produced per block, and qc=0's
    attention batches are emitted as soon as their k-tiles project.
  - Attention per (qc, kc) batch: the two heads' S^T matmuls (K=64)
    row-pack into PE halves via base_partition 0/64 and run
    concurrently into one [128, 1024] PSUM tile.
  - exp is split across two engines: ~55%% of batches run ACT
    ACTIVATE(Exp) -> fp16; the rest run a fused Schraudolph bit-trick
    on DVE (one tensor_scalar: i16 = rint(s*184.66 + 15315.5), whose
    bits ARE fp16 exp(s/8), ~1.8%% sigma, cancels in softmax norm).
  - E@V: per (kc, h) one [128(k), 80] V-tile ([V_h | ones | 15 pad])
    accumulates O^T (+ row sums in row 64) over kc into [80, 512] PSUM.
  - Output: transpose O^T via xbar DMA-transpose (PE transpose on the
    final q chunk), reciprocal of the row-sum column, per-partition
    scale, DMA out.
"""

import numpy as np

N_CORES = 8
S_FULL = 4096
D_MODEL = 512
HEAD = 64

# Schraudolph constants for fp16-targeted exp(s/8) bit trick
SCHR_ALPHA = 0.125 * 1024.0 / float(np.log(2.0))   # 184.6643...
SCHR_BETA = 15315.5

N_ACT = 17   # of 32 kc batches per qc, how many exp on ACT (rest on DVE)

_cached = {}


def _on_act(kc):
    return (kc * N_ACT) // 32 != ((kc + 1) * N_ACT) // 32


def build_nc(S=S_FULL):
    import concourse.bass as bass
    from concourse import bacc
    import concourse.mybir as mybir
    import concourse.tile as tile
    from concourse.masks import make_identity
    f32 = mybir.dt.float32
    f16 = mybir.dt.float16
    i16 = mybir.dt.int16
    AF = mybir.ActivationFunctionType
    ALU = mybir.AluOpType

    D = D_MODEL
    n_qc = S // 512     # 512-wide query chunks
    n_kc = S // 128     # 128-wide key tiles

    nc = bacc.Bacc()

    xT = nc.dram_tensor("xT", [D, S], f16, kind="ExternalInput")
    wqT = nc.dram_tensor("wqT", [D, 128], f16, kind="ExternalInput")
    wkT = nc.dram_tensor("wkT", [D, 128], f16, kind="ExternalInput")
    wvT = nc.dram_tensor("wvT", [D, 160], f16, kind="ExternalInput")
    bq = nc.dram_tensor("bq", [128, 1], f32, kind="ExternalInput")
    bk = nc.dram_tensor("bk", [128, 1], f32, kind="ExternalInput")
    bvb = nc.dram_tensor("bvb", [128, 160], f32, kind="ExternalInput")
    out = nc.dram_tensor("out", [S, 128], f32, kind="ExternalOutput")

    with tile.TileContext(nc) as tc:
        with (
            tc.tile_pool(name="consts", bufs=1) as consts,
            tc.tile_pool(name="persist", bufs=1) as persist,
            tc.tile_pool(name="stp", bufs=1, space="PSUM") as stp,
            tc.tile_pool(name="pop", bufs=1, space="PSUM") as pop,
            tc.tile_pool(name="etp", bufs=4) as etp,
            tc.tile_pool(name="outp", bufs=2) as outp,
        ):
            ident = consts.tile([128, 128], f16, name="ident")
            make_identity(nc, ident)
            wq_sb = consts.tile([128, 512], f16, name="wq_sb")
            wk_sb = consts.tile([128, 512], f16, name="wk_sb")
            wv_sb = consts.tile([128, 640], f16, name="wv_sb")
            bq_sb = consts.tile([128, 1], f32, name="bq_sb")
            bk_sb = consts.tile([128, 1], f32, name="bk_sb")
            bvb_sb = consts.tile([128, 160], f32, name="bvb_sb")
            xt = [persist.tile([128, S], f16, name=f"xt{i}") for i in range(4)]
            qt = persist.tile([128, S], f16, name="qt")
            kt = persist.tile([128, S], f16, name="kt")
            # v1[:, kc*160 + h*80 : +80] = [V_h(64) | ones | 15 zero pad]
            v1 = persist.tile([128, n_kc * 160], f16, name="v1")

            for dc in range(4):
                r = slice(dc * 128, (dc + 1) * 128)
                nc.sync.dma_start(wq_sb[:, dc * 128:(dc + 1) * 128], wqT[r, :])
                nc.sync.dma_start(wk_sb[:, dc * 128:(dc + 1) * 128], wkT[r, :])
                nc.sync.dma_start(wv_sb[:, dc * 160:(dc + 1) * 160], wvT[r, :])
            nc.sync.dma_start(bq_sb[:], bq[:, :])
            nc.sync.dma_start(bk_sb[:], bk[:, :])
            nc.sync.dma_start(bvb_sb[:], bvb[:, :])

            # ---------------- shared PSUM ring (3 x [128,1024] = 6 banks) --
            # Projections and attention score tiles rotate through the same
            # three tags via a single global counter, so the ring stays
            # round-robin even while qc0's attention interleaves with the
            # tail of the projections.
            st_ctr = [0]

            def st_alloc(name):
                t = stp.tile([128, 1024], f32, name=name,
                             tag=f"st{st_ctr[0] % 3}")
                st_ctr[0] += 1
                return t

            # ---------------- projection emitters ----------------
            def emit_k_chunk(cs):
                pk = st_alloc("pk")
                for dc in range(4):
                    nc.tensor.matmul(
                        pk[:, 0:512],
                        lhsT=wk_sb[:, dc * 128:(dc + 1) * 128],
                        rhs=xt[dc][:, cs],
                        start=(dc == 0),
                        stop=(dc == 3),
                    )
                nc.scalar.activation(kt[:, cs], pk[:, 0:512], AF.Identity,
                                     bias=bk_sb[:])

            def emit_q_chunk(cs):
                pq = st_alloc("pq")
                for dc in range(4):
                    nc.tensor.matmul(
                        pq[:, 0:512],
                        lhsT=wq_sb[:, dc * 128:(dc + 1) * 128],
                        rhs=xt[dc][:, cs],
                        start=(dc == 0),
                        stop=(dc == 3),
                    )
                nc.scalar.activation(qt[:, cs], pq[:, 0:512], AF.Identity,
                                     bias=bq_sb[:])

            def emit_v_ktile(kc):
                ks = slice(kc * 128, (kc + 1) * 128)
                pv = st_alloc("pv")
                for dc in range(4):
                    nc.tensor.matmul(
                        pv[:, 0:160],
                        lhsT=xt[dc][:, ks],
                        rhs=wv_sb[:, dc * 160:(dc + 1) * 160],
                        start=(dc == 0),
                        stop=(dc == 3),
                    )
                nc.vector.tensor_add(
                    v1[:, kc * 160:(kc + 1) * 160], pv[:, 0:160], bvb_sb[:]
                )

            # ---------------- attention emitters ----------------
            pend_ev = [None]     # deferred EV pair (kc=31 of previous qc)
            pend_norm = [None]   # deferred epilogue of previous qc
            cur = {}             # per-qc state: po tiles, et ring

            def emit_ev(qc, kc, et):
                for h in range(2):
                    nc.tensor.matmul(
                        cur["po"][h][0:80, :],
                        lhsT=v1[:, kc * 160 + h * 80: kc * 160 + (h + 1) * 80],
                        rhs=et[:, h * 512:(h + 1) * 512],
                        start=(kc == 0),
                        stop=(kc == n_kc - 1),
                    )

            def emit_norm(po, qc):
                last = qc == n_qc - 1
                res = [
                    outp.tile([128, 128], f32, name=f"res{t}", tag=f"res{t}")
                    for t in range(4)
                ]
                for h in range(2):
                    ot = outp.tile([128, 512], f16, name="ot", tag=f"ot{h}")
                    nc.scalar.activation(ot[0:80, :], po[h][0:80, :], AF.Identity)
                    for t in range(4):
                        if last:
                            # PE transpose (xbar DMA transposes would
                            # serialize on the exposed kernel tail)
                            pt = stp.tile([128, 65], f16, name="pt",
                                          tag=f"st{st_ctr[0] % 3}")
                            st_ctr[0] += 1
                            nc.tensor.transpose(
                                pt[:],
                                ot[0:65, t * 128:(t + 1) * 128],
                                ident[0:65, 0:65],
                            )
                            src = pt
                        else:
                            tp = outp.tile([128, 80], f16, name="tp", tag="tp")
                            nc.sync.dma_start_transpose(
                                tp[:], ot[0:80, t * 128:(t + 1) * 128]
                            )
                            src = tp
                        rcp = outp.tile([128, 1], f32, name="rcp", tag="rcp")
                        nc.vector.reciprocal(rcp[:], src[:, 64:65])
                        nc.vector.tensor_scalar_mul(
                            res[t][:, h * 64:(h + 1) * 64], src[:, 0:64], rcp[:]
                        )
                for t in range(4):
                    nc.sync.dma_start(
                        out[qc * 512 + t * 128: qc * 512 + (t + 1) * 128, :],
                        res[t][:],
                    )

            def attn_batches(qc, kc_lo, kc_hi):
                qs = slice(qc * 512, (qc + 1) * 512)
                for kc in range(kc_lo, kc_hi):
                    if kc == 0:
                        cur["et"] = {}
                    st = st_alloc("st")
                    for h in range(2):
                        hp = slice(h * 64, (h + 1) * 64)
                        nc.tensor.matmul(
                            st[:, h * 512:(h + 1) * 512],
                            lhsT=kt[hp, kc * 128:(kc + 1) * 128],
                            rhs=qt[hp, qs],
                            start=True,
                            stop=True,
                        )
                    if kc == 0:
                        # previous qc's tail EV + epilogue, deferred past
                        # this qc's first scores so the PE stays fed
                        if pend_ev[0] is not None:
                            pend_ev[0]()
                            pend_ev[0] = None
                        if pend_norm[0] is not None:
                            pend_norm[0]()
                            pend_norm[0] = None
                    et = etp.tile([128, 1024], f16, name="et", tag="et")
                    if _on_act(kc):
                        nc.scalar.activation(et[:], st[:], AF.Exp, scale=0.125)
                    else:
                        nc.vector.tensor_scalar(
                            et[:].bitcast(i16), st[:],
                            SCHR_ALPHA, SCHR_BETA, ALU.mult, ALU.add,
                        )
                    cur["et"][kc] = et
                    if kc == 2:
                        # allocate this qc's O^T accumulators only now: the
                        # previous qc's epilogue reads (same PSUM banks,
                        # bufs=1) are already emitted above, so the WAR
                        # dependency is tracked.
                        cur["po"] = [
                            pop.tile([128, 512], f32, name=f"po{h}",
                                     tag=f"po{h}")
                            for h in range(2)
                        ]
                    if kc >= 2:
                        emit_ev(qc, kc - 2, cur["et"].pop(kc - 2))
                if kc_hi == n_kc:
                    emit_ev(qc, n_kc - 2, cur["et"].pop(n_kc - 2))
                    et31 = cur["et"].pop(n_kc - 1)
                    po = cur["po"]
                    pend_ev[0] = lambda qc=qc, et=et31: emit_ev(qc, n_kc - 1, et)
                    pend_norm[0] = lambda po=po, qc=qc: emit_norm(po, qc)

            # ---------------- emission: blocks + pipelined qc0 ----------
            blocks = [(0, 512), (512, 512), (1024, 1024), (2048, 1024),
                      (3072, 1024)]
            q_sched = {0: [0], 1: [1], 2: [2, 3], 3: [4, 5], 4: [6, 7]}
            att_sched = {1: (0, 8), 2: (8, 16), 3: (16, 24), 4: (24, 32)}
            for bi, (boff, blk) in enumerate(blocks):
                for dc in range(4):
                    if bi == 0:
                        for hh in range(2):
                            cs = slice(boff + hh * 256, boff + (hh + 1) * 256)
                            nc.sync.dma_start(xt[dc][:, cs],
                                              xT[dc * 128:(dc + 1) * 128, cs])
                    else:
                        cs = slice(boff, boff + blk)
                        nc.sync.dma_start(xt[dc][:, cs],
                                          xT[dc * 128:(dc + 1) * 128, cs])
                for half in range(blk // 512):
                    hs = slice(boff + half * 512, boff + (half + 1) * 512)
                    emit_k_chunk(hs)
                for kci in range(boff // 128, (boff + blk) // 128):
                    emit_v_ktile(kci)
                for qi in q_sched[bi]:
                    emit_q_chunk(slice(qi * 512, (qi + 1) * 512))
                if bi in att_sched:
                    attn_batches(0, *att_sched[bi])

            for qc in range(1, n_qc):
                attn_batches(qc, 0, n_kc)
            pend_ev[0]()
            pend_norm[0]()
    return nc


def _shard_inputs(x, Wq, bq, Wk, bk, Wv, bv):
    """Build the 8 per-core input maps from full inputs."""
    x = np.asarray(x, dtype=np.float32)
    in_maps = []
    for c in range(N_CORES):
        b, pair = c // 4, c % 4
        rows = slice(pair * 128, (pair + 1) * 128)
        wq_s = np.asarray(Wq)[rows, :].astype(np.float32)
        wk_s = np.asarray(Wk)[rows, :].astype(np.float32)
        wv_s = np.asarray(Wv)[rows, :].astype(np.float32)
        bq_s = np.asarray(bq)[rows].astype(np.float32)
        bk_s = np.asarray(bk)[rows].astype(np.float32)
        bv_s = np.asarray(bv)[rows].astype(np.float32)

        wvT = np.zeros((D_MODEL, 160), np.float32)
        wvT[:, 0:64] = wv_s[0:64].T
        wvT[:, 80:144] = wv_s[64:128].T
        bvb = np.zeros((128, 160), np.float32)
        bvb[:, 0:64] = bv_s[0:64]
        bvb[:, 64] = 1.0
        bvb[:, 80:144] = bv_s[64:128]
        bvb[:, 144] = 1.0

        in_maps.append({
            "xT": np.ascontiguousarray(x[c // 4].T).astype(np.float16),
            "wqT": np.ascontiguousarray(wq_s.T).astype(np.float16),
            "wkT": np.ascontiguousarray(wk_s.T).astype(np.float16),
            "wvT": wvT.astype(np.float16),
            "bq": bq_s.reshape(128, 1).copy(),
            "bk": bk_s.reshape(128, 1).copy(),
            "bvb": bvb,
        })
    return in_maps


def _gather(results):
    B, S, D = 2, S_FULL, D_MODEL
    out = np.empty((B, S, D), np.float32)
    for c in range(N_CORES):
        b, pair = c // 4, c % 4
        out[b, :, pair * 128:(pair + 1) * 128] = results[c]["out"]
    return out


def _install_profile_hook():
    """Provide antenv.axon_hooks (missing in this image) so that
    run_bass_kernel_spmd(trace=True) can capture NTFF profiles, using the
    same ctypes path trn_boot.py would have registered."""
    import sys, types, ctypes, contextlib

    if "antenv.axon_hooks" in sys.modules:
        return
    so_path = "/opt/axon/libaxon_pjrt.so"
    mod = types.ModuleType("antenv.axon_hooks")
    state = {"hook": None}
    mod.set_axon_ntff_profile_hook = lambda h: state.__setitem__("hook", h)
    mod.get_axon_ntff_profile_hook = lambda: state["hook"]
    sys.modules["antenv.axon_hooks"] = mod
    try:
        lib = ctypes.CDLL(so_path)
        if not hasattr(lib, "axon_start_nrt_profile"):
            return
        lib.axon_start_nrt_profile.argtypes = [
            ctypes.POINTER(ctypes.c_int64), ctypes.c_size_t]
        lib.axon_start_nrt_profile.restype = ctypes.c_int64
        lib.axon_stop_nrt_profile.argtypes = [ctypes.c_char_p]
        lib.axon_stop_nrt_profile.restype = ctypes.c_int64

        @contextlib.contextmanager
        def _hook(output_dir, device_ids):
            import jax
            jax.devices()
            if device_ids:
                ids = (ctypes.c_int64 * len(device_ids))(*device_ids)
                rc = lib.axon_start_nrt_profile(ids, len(device_ids))
            else:
                rc = lib.axon_start_nrt_profile(None, 0)
            if rc != 0:
                raise RuntimeError(f"axon_start_nrt_profile rc={rc}")
            try:
                yield
            finally:
                n = lib.axon_stop_nrt_profile(str(output_dir).encode())
                print(f"profile: {n} file(s) written to {output_dir}")

        state["hook"] = _hook
    except OSError:
        pass


def kernel(x, Wq, bq, Wk, bk, Wv, bv, trace=False):
    from concourse.bass_utils import run_bass_kernel_spmd

    if trace:
        _install_profile_hook()
    if "nc" not in _cached:
        nc = build_nc(S_FULL)
        nc.finalize()
        _cached["nc"] = nc
    nc = _cached["nc"]
    in_maps = _shard_inputs(x, Wq, bq, Wk, bk, Wv, bv)
    r = run_bass_kernel_spmd(nc, in_maps, list(range(N_CORES)), trace=trace)
    _cached["last_results"] = r
    return _gather(r.results)


# revision 9
# speedup vs baseline: 1.1723x; 1.1723x over previous
"""Multi-head attention (B=2, S=4096, D=512, H=8) on 8 NeuronCores.

Sharding: data-parallel on batch x head-pair-parallel.  Core c handles
batch b = c//4 and heads (2*(c%4), 2*(c%4)+1).  Each core computes its
[4096, 128] slice of the output; the host scatters inputs / gathers
outputs.

Per-core kernel (Bass/Tile), fp16 operands (fp32 PSUM accumulate):
  - Projections pipelined with the first query chunk's attention: xT
    blocks DMA in; K^T / Q^T ([128, S], head dims on partitions, bias
    added on ACT) and V ([k, 80]-per-(ktile,head) tiles with a ones
    column, bias added on DVE) are produced per block, and qc=0's
    attention batches are emitted as soon as their k-tiles project.
  - Attention per (qc, kc) batch: the two heads' S^T matmuls (K=64)
    row-pack into PE halves via base_partition 0/64 and run
    concurrently into one [128, 1024] PSUM tile.
  - exp is split across two engines: ~55%% of batches run ACT
    ACTIVATE(Exp) -> fp16; the rest run a fused Schraudolph bit-trick
    on DVE (one tensor_scalar: i16 = rint(s*184.66 + 15315.5), whose
    bits ARE fp16 exp(s/8), ~1.8%% sigma, cancels in softmax norm).
  - E@V: per (kc, h) one [128(k), 80] V-tile ([V_h | ones | 15 pad])
    accumulates O^T (+ row sums in row 64) over kc into [80, 512] PSUM.
  - Output: transpose O^T via xbar DMA-transpose (PE transpose on the
    final q chunk), reciprocal of the row-sum column, per-partition
    scale, DMA out.
"""

import numpy as np

N_CORES = 8
S_FULL = 4096
D_MODEL = 512
HEAD = 64

# Schraudolph constants for fp16-targeted exp(s/8) bit trick
SCHR_ALPHA = 0.125 * 1024.0 / float(np.log(2.0))   # 184.6643...
SCHR_BETA = 15315.5

N_ACT = 17   # of 32 kc batches per qc, how many exp on ACT (rest on DVE)

_cached = {}


def _on_act(kc):
    # kc 30 on ACT and kc 31 on DVE so the tail EV + epilogue chain at the
    # qc boundary overlaps both engines; 16 of kc 0..29 spread on ACT.
    if kc == 30:
        return True
    if kc == 31:
        return False
    return (kc * 16) // 30 != ((kc + 1) * 16) // 30


def build_nc(S=S_FULL):
    import concourse.bass as bass
    from concourse import bacc
    import concourse.mybir as mybir
    import concourse.tile as tile
    from concourse.masks import make_identity
    f32 = mybir.dt.float32
    f16 = mybir.dt.float16
    i16 = mybir.dt.int16
    AF = mybir.ActivationFunctionType
    ALU = mybir.AluOpType

    D = D_MODEL
    n_qc = S // 512     # 512-wide query chunks
    n_kc = S // 128     # 128-wide key tiles

    nc = bacc.Bacc()

    xT = nc.dram_tensor("xT", [D, S], f16, kind="ExternalInput")
    wqT = nc.dram_tensor("wqT", [D, 128], f16, kind="ExternalInput")
    wkT = nc.dram_tensor("wkT", [D, 128], f16, kind="ExternalInput")
    wvT = nc.dram_tensor("wvT", [D, 256], f16, kind="ExternalInput")
    bq = nc.dram_tensor("bq", [128, 1], f32, kind="ExternalInput")
    bk = nc.dram_tensor("bk", [128, 1], f32, kind="ExternalInput")
    bvb = nc.dram_tensor("bvb", [128, 256], f32, kind="ExternalInput")
    out = nc.dram_tensor("out", [S, 128], f32, kind="ExternalOutput")

    with tile.TileContext(nc) as tc:
        with (
            tc.tile_pool(name="consts", bufs=1) as consts,
            tc.tile_pool(name="persist", bufs=1) as persist,
            tc.tile_pool(name="stp", bufs=1, space="PSUM") as stp,
            tc.tile_pool(name="pop", bufs=1, space="PSUM") as pop,
            tc.tile_pool(name="etp", bufs=4) as etp,
            tc.tile_pool(name="outp", bufs=2) as outp,
        ):
            ident = consts.tile([128, 128], f16, name="ident")
            make_identity(nc, ident)
            wq_sb = consts.tile([128, 512], f16, name="wq_sb")
            wk_sb = consts.tile([128, 512], f16, name="wk_sb")
            wv_sb = consts.tile([128, 1024], f16, name="wv_sb")
            bq_sb = consts.tile([128, 1], f32, name="bq_sb")
            bk_sb = consts.tile([128, 1], f32, name="bk_sb")
            bvb_sb = consts.tile([128, 256], f32, name="bvb_sb")
            xt = [persist.tile([128, S], f16, name=f"xt{i}") for i in range(4)]
            qt = persist.tile([128, S], f16, name="qt")
            kt = persist.tile([128, S], f16, name="kt")
            # v1[:, kc*256 + h*128 : +128] = [V_h(64) | ones | 63 zero pad]
            # (full 128-col weight tiles keep the EV LDWEIGHTS on the FWL path)
            v1 = persist.tile([128, n_kc * 256], f16, name="v1")

            for dc in range(4):
                r = slice(dc * 128, (dc + 1) * 128)
                nc.sync.dma_start(wq_sb[:, dc * 128:(dc + 1) * 128], wqT[r, :])
                nc.sync.dma_start(wk_sb[:, dc * 128:(dc + 1) * 128], wkT[r, :])
                nc.sync.dma_start(wv_sb[:, dc * 256:(dc + 1) * 256], wvT[r, :])
            nc.sync.dma_start(bq_sb[:], bq[:, :])
            nc.sync.dma_start(bk_sb[:], bk[:, :])
            nc.sync.dma_start(bvb_sb[:], bvb[:, :])

            # ---------------- shared PSUM ring (3 x [128,1024] = 6 banks) --
            # Projections and attention score tiles rotate through the same
            # three tags via a single global counter, so the ring stays
            # round-robin even while qc0's attention interleaves with the
            # tail of the projections.
            st_ctr = [0]

            def st_alloc(name):
                t = stp.tile([128, 1024], f32, name=name,
                             tag=f"st{st_ctr[0] % 3}")
                st_ctr[0] += 1
                return t

            # ---------------- projection emitters ----------------
            def emit_k_chunk(cs):
                pk = st_alloc("pk")
                for dc in range(4):
                    nc.tensor.matmul(
                        pk[:, 0:512],
                        lhsT=wk_sb[:, dc * 128:(dc + 1) * 128],
                        rhs=xt[dc][:, cs],
                        start=(dc == 0),
                        stop=(dc == 3),
                    )
                nc.scalar.activation(kt[:, cs], pk[:, 0:512], AF.Identity,
                                     bias=bk_sb[:])

            def emit_q_chunk(cs):
                pq = st_alloc("pq")
                for dc in range(4):
                    nc.tensor.matmul(
                        pq[:, 0:512],
                        lhsT=wq_sb[:, dc * 128:(dc + 1) * 128],
                        rhs=xt[dc][:, cs],
                        start=(dc == 0),
                        stop=(dc == 3),
                    )
                nc.scalar.activation(qt[:, cs], pq[:, 0:512], AF.Identity,
                                     bias=bq_sb[:])

            def emit_v_ktile(kc):
                ks = slice(kc * 128, (kc + 1) * 128)
                pv = st_alloc("pv")
                for dc in range(4):
                    nc.tensor.matmul(
                        pv[:, 0:256],
                        lhsT=xt[dc][:, ks],
                        rhs=wv_sb[:, dc * 256:(dc + 1) * 256],
                        start=(dc == 0),
                        stop=(dc == 3),
                    )
                nc.vector.tensor_add(
                    v1[:, kc * 256:(kc + 1) * 256], pv[:, 0:256], bvb_sb[:]
                )

            # ---------------- attention emitters ----------------
            pend_tail = [None]   # deferred kc=31 EV + epilogue of prev qc
            cur = {}             # per-qc state: po tiles, et ring

            def emit_ev(qc, kc, et):
                for h in range(2):
                    nc.tensor.matmul(
                        cur["po"][h][:],
                        lhsT=v1[:, kc * 256 + h * 128: kc * 256 + (h + 1) * 128],
                        rhs=et[:, h * 512:(h + 1) * 512],
                        start=(kc == 0),
                        stop=(kc == n_kc - 1),
                    )

            def emit_tail(po, qc, et31):
                # kc=31 EV and the PSUM->SBUF copy interleaved per head, so
                # the next qc's first EV (which reuses these PSUM banks) is
                # unblocked as early as possible.
                last = qc == n_qc - 1
                res = [
                    outp.tile([128, 128], f32, name=f"res{t}", tag=f"res{t}")
                    for t in range(4)
                ]
                for h in range(2):
                    nc.tensor.matmul(
                        po[h][:],
                        lhsT=v1[:, (n_kc - 1) * 256 + h * 128:
                                (n_kc - 1) * 256 + (h + 1) * 128],
                        rhs=et31[:, h * 512:(h + 1) * 512],
                        start=False,
                        stop=True,
                    )
                    ot = outp.tile([128, 512], f16, name="ot", tag=f"ot{h}")
                    nc.scalar.activation(ot[0:80, :], po[h][0:80, :], AF.Identity)
                    for t in range(4):
                        if last:
                            # PE transpose (xbar DMA transposes would
                            # serialize on the exposed kernel tail)
                            pt = stp.tile([128, 65], f16, name="pt",
                                          tag=f"st{st_ctr[0] % 3}")
                            st_ctr[0] += 1
                            nc.tensor.transpose(
                                pt[:],
                                ot[0:65, t * 128:(t + 1) * 128],
                                ident[0:65, 0:65],
                            )
                            src = pt
                        else:
                            tp = outp.tile([128, 80], f16, name="tp", tag="tp")
                            nc.sync.dma_start_transpose(
                                tp[:], ot[0:80, t * 128:(t + 1) * 128]
                            )
                            src = tp
                        rcp = outp.tile([128, 1], f32, name="rcp", tag="rcp")
                        nc.vector.reciprocal(rcp[:], src[:, 64:65])
                        nc.vector.tensor_scalar_mul(
                            res[t][:, h * 64:(h + 1) * 64], src[:, 0:64], rcp[:]
                        )
                for t in range(4):
                    nc.sync.dma_start(
                        out[qc * 512 + t * 128: qc * 512 + (t + 1) * 128, :],
                        res[t][:],
                    )

            def attn_batches(qc, kc_lo, kc_hi):
                qs = slice(qc * 512, (qc + 1) * 512)
                for kc in range(kc_lo, kc_hi):
                    if kc == 0:
                        cur["et"] = {}
                    st = st_alloc("st")
                    for h in range(2):
                        hp = slice(h * 64, (h + 1) * 64)
                        nc.tensor.matmul(
                            st[:, h * 512:(h + 1) * 512],
                            lhsT=kt[hp, kc * 128:(kc + 1) * 128],
                            rhs=qt[hp, qs],
                            start=True,
                            stop=True,
                        )
                    if kc == 0:
                        # previous qc's tail EV + epilogue, deferred past
                        # this qc's first scores so the PE stays fed
                        if pend_tail[0] is not None:
                            pend_tail[0]()
                            pend_tail[0] = None
                    et = etp.tile([128, 1024], f16, name="et", tag="et")
                    if _on_act(kc):
                        nc.scalar.activation(et[:], st[:], AF.Exp, scale=0.125)
                    else:
                        nc.vector.tensor_scalar(
                            et[:].bitcast(i16), st[:],
                            SCHR_ALPHA, SCHR_BETA, ALU.mult, ALU.add,
                        )
                    cur["et"][kc] = et
                    if kc == 2:
                        # allocate this qc's O^T accumulators only now: the
                        # previous qc's epilogue reads (same PSUM banks,
                        # bufs=1) are already emitted above, so the WAR
                        # dependency is tracked.
                        cur["po"] = [
                            pop.tile([128, 512], f32, name=f"po{h}",
                                     tag=f"po{h}")
                            for h in range(2)
                        ]
                    if kc >= 2:
                        emit_ev(qc, kc - 2, cur["et"].pop(kc - 2))
                if kc_hi == n_kc:
                    emit_ev(qc, n_kc - 2, cur["et"].pop(n_kc - 2))
                    et31 = cur["et"].pop(n_kc - 1)
                    po = cur["po"]
                    pend_tail[0] = (
                        lambda po=po, qc=qc, et=et31: emit_tail(po, qc, et)
                    )

            # ---------------- emission: blocks + pipelined qc0 ----------
            blocks = [(0, 512), (512, 512), (1024, 1024), (2048, 1024),
                      (3072, 1024)]
            q_sched = {0: [0], 1: [1], 2: [2, 3], 3: [4, 5], 4: [6, 7]}
            att_sched = {1: (0, 8), 2: (8, 16), 3: (16, 24), 4: (24, 32)}
            for bi, (boff, blk) in enumerate(blocks):
                for dc in range(4):
                    if bi == 0:
                        for hh in range(2):
                            cs = slice(boff + hh * 256, boff + (hh + 1) * 256)
                            nc.sync.dma_start(xt[dc][:, cs],
                                              xT[dc * 128:(dc + 1) * 128, cs])
                    else:
                        cs = slice(boff, boff + blk)
                        nc.sync.dma_start(xt[dc][:, cs],
                                          xT[dc * 128:(dc + 1) * 128, cs])
                for half in range(blk // 512):
                    hs = slice(boff + half * 512, boff + (half + 1) * 512)
                    emit_k_chunk(hs)
                for kci in range(boff // 128, (boff + blk) // 128):
                    emit_v_ktile(kci)
                for qi in q_sched[bi]:
                    emit_q_chunk(slice(qi * 512, (qi + 1) * 512))
                if bi in att_sched:
                    attn_batches(0, *att_sched[bi])

            for qc in range(1, n_qc):
                attn_batches(qc, 0, n_kc)
            pend_tail[0]()
    return nc


def _shard_inputs(x, Wq, bq, Wk, bk, Wv, bv):
    """Build the 8 per-core input maps from full inputs."""
    x = np.asarray(x, dtype=np.float32)
    in_maps = []
    for c in range(N_CORES):
        b, pair = c // 4, c % 4
        rows = slice(pair * 128, (pair + 1) * 128)
        wq_s = np.asarray(Wq)[rows, :].astype(np.float32)
        wk_s = np.asarray(Wk)[rows, :].astype(np.float32)
        wv_s = np.asarray(Wv)[rows, :].astype(np.float32)
        bq_s = np.asarray(bq)[rows].astype(np.float32)
        bk_s = np.asarray(bk)[rows].astype(np.float32)
        bv_s = np.asarray(bv)[rows].astype(np.float32)

        wvT = np.zeros((D_MODEL, 256), np.float32)
        wvT[:, 0:64] = wv_s[0:64].T
        wvT[:, 128:192] = wv_s[64:128].T
        bvb = np.zeros((128, 256), np.float32)
        bvb[:, 0:64] = bv_s[0:64]
        bvb[:, 64] = 1.0
        bvb[:, 128:192] = bv_s[64:128]
        bvb[:, 192] = 1.0

        in_maps.append({
            "xT": np.ascontiguousarray(x[c // 4].T).astype(np.float16),
            "wqT": np.ascontiguousarray(wq_s.T).astype(np.float16),
            "wkT": np.ascontiguousarray(wk_s.T).astype(np.float16),
            "wvT": wvT.astype(np.float16),
            "bq": bq_s.reshape(128, 1).copy(),
            "bk": bk_s.reshape(128, 1).copy(),
            "bvb": bvb,
        })
    return in_maps


def _gather(results):
    B, S, D = 2, S_FULL, D_MODEL
    out = np.empty((B, S, D), np.float32)
    for c in range(N_CORES):
        b, pair = c // 4, c % 4
        out[b, :, pair * 128:(pair + 1) * 128] = results[c]["out"]
    return out


def _install_profile_hook():
    """Provide antenv.axon_hooks (missing in this image) so that
    run_bass_kernel_spmd(trace=True) can capture NTFF profiles, using the
    same ctypes path trn_boot.py would have registered."""
    import sys, types, ctypes, contextlib

    if "antenv.axon_hooks" in sys.modules:
        return
    so_path = "/opt/axon/libaxon_pjrt.so"
    mod = types.ModuleType("antenv.axon_hooks")
    state = {"hook": None}
    mod.set_axon_ntff_profile_hook = lambda h: state.__setitem__("hook", h)
    mod.get_axon_ntff_profile_hook = lambda: state["hook"]
    sys.modules["antenv.axon_hooks"] = mod
    try:
        lib = ctypes.CDLL(so_path)
        if not hasattr(lib, "axon_start_nrt_profile"):
            return
        lib.axon_start_nrt_profile.argtypes = [
            ctypes.POINTER(ctypes.c_int64), ctypes.c_size_t]
        lib.axon_start_nrt_profile.restype = ctypes.c_int64
        lib.axon_stop_nrt_profile.argtypes = [ctypes.c_char_p]
        lib.axon_stop_nrt_profile.restype = ctypes.c_int64

        @contextlib.contextmanager
        def _hook(output_dir, device_ids):
            import jax
            jax.devices()
            if device_ids:
                ids = (ctypes.c_int64 * len(device_ids))(*device_ids)
                rc = lib.axon_start_nrt_profile(ids, len(device_ids))
            else:
                rc = lib.axon_start_nrt_profile(None, 0)
            if rc != 0:
                raise RuntimeError(f"axon_start_nrt_profile rc={rc}")
            try:
                yield
            finally:
                n = lib.axon_stop_nrt_profile(str(output_dir).encode())
                print(f"profile: {n} file(s) written to {output_dir}")

        state["hook"] = _hook
    except OSError:
        pass


def kernel(x, Wq, bq, Wk, bk, Wv, bv, trace=False):
    from concourse.bass_utils import run_bass_kernel_spmd

    if trace:
        _install_profile_hook()
    if "nc" not in _cached:
        nc = build_nc(S_FULL)
        nc.finalize()
        _cached["nc"] = nc
    nc = _cached["nc"]
    in_maps = _shard_inputs(x, Wq, bq, Wk, bk, Wv, bv)
    r = run_bass_kernel_spmd(nc, in_maps, list(range(N_CORES)), trace=trace)
    _cached["last_results"] = r
    return _gather(r.results)


# revision 10
# speedup vs baseline: 1.2111x; 1.0332x over previous
"""Multi-head attention (B=2, S=4096, D=512, H=8) on 8 NeuronCores.

Sharding: data-parallel on batch x head-pair-parallel.  Core c handles
batch b = c//4 and heads (2*(c%4), 2*(c%4)+1).  Each core computes its
[4096, 128] slice of the output; the host scatters inputs / gathers
outputs.

Per-core kernel (Bass/Tile), fp16 operands (fp32 PSUM accumulate):
  - Projections pipelined with the first query chunk's attention: xT
    blocks DMA in; K^T / Q^T ([128, S], head dims on partitions, bias
    added on ACT) and V ([k, 80]-per-(ktile,head) tiles with a ones
    column, bias added on DVE) are produced per block, and qc=0's
    attention batches are emitted as soon as their k-tiles project.
  - Attention per (qc, kc) batch: the two heads' S^T matmuls (K=64)
    row-pack into PE halves via base_partition 0/64 and run
    concurrently into one [128, 1024] PSUM tile.
  - exp is split across two engines: ~55%% of batches run ACT
    ACTIVATE(Exp) -> fp16; the rest run a fused Schraudolph bit-trick
    on DVE (one tensor_scalar: i16 = rint(s*184.66 + 15315.5), whose
    bits ARE fp16 exp(s/8), ~1.8%% sigma, cancels in softmax norm).
  - E@V: per (kc, h) one [128(k), 80] V-tile ([V_h | ones | 15 pad])
    accumulates O^T (+ row sums in row 64) over kc into [80, 512] PSUM.
  - Output: transpose O^T via xbar DMA-transpose (PE transpose on the
    final q chunk), reciprocal of the row-sum column, per-partition
    scale, DMA out.
"""

import numpy as np

N_CORES = 8
S_FULL = 4096
D_MODEL = 512
HEAD = 64

# Schraudolph constants for fp16-targeted exp(s/8) bit trick
SCHR_ALPHA = 0.125 * 1024.0 / float(np.log(2.0))   # 184.6643...
SCHR_BETA = 15315.5

N_ACT = 17   # of 32 kc batches per qc, how many exp on ACT (rest on DVE)

_cached = {}


def _on_act(kc):
    # kc 30 on ACT and kc 31 on DVE so the tail EV + epilogue chain at the
    # qc boundary overlaps both engines; 16 of kc 0..29 spread on ACT.
    if kc == 30:
        return True
    if kc == 31:
        return False
    return (kc * 16) // 30 != ((kc + 1) * 16) // 30


def build_nc(S=S_FULL):
    import concourse.bass as bass
    from concourse import bacc
    import concourse.mybir as mybir
    import concourse.tile as tile
    from concourse.masks import make_identity
    f32 = mybir.dt.float32
    f16 = mybir.dt.float16
    i16 = mybir.dt.int16
    AF = mybir.ActivationFunctionType
    ALU = mybir.AluOpType

    D = D_MODEL
    n_qc = S // 512     # 512-wide query chunks
    n_kc = S // 128     # 128-wide key tiles

    nc = bacc.Bacc()

    xT = nc.dram_tensor("xT", [D, S], f16, kind="ExternalInput")
    wqT = nc.dram_tensor("wqT", [D, 128], f16, kind="ExternalInput")
    wkT = nc.dram_tensor("wkT", [D, 128], f16, kind="ExternalInput")
    wvT = nc.dram_tensor("wvT", [D, 256], f16, kind="ExternalInput")
    bq = nc.dram_tensor("bq", [128, 1], f32, kind="ExternalInput")
    bk = nc.dram_tensor("bk", [128, 1], f32, kind="ExternalInput")
    bvb = nc.dram_tensor("bvb", [128, 256], f32, kind="ExternalInput")
    out = nc.dram_tensor("out", [S, 128], f32, kind="ExternalOutput")

    with tile.TileContext(nc) as tc:
        with (
            tc.tile_pool(name="consts", bufs=1) as consts,
            tc.tile_pool(name="persist", bufs=1) as persist,
            tc.tile_pool(name="stp", bufs=1, space="PSUM") as stp,
            tc.tile_pool(name="pop", bufs=1, space="PSUM") as pop,
            tc.tile_pool(name="etp", bufs=5) as etp,
            tc.tile_pool(name="outp", bufs=2) as outp,
        ):
            ident = consts.tile([128, 128], f16, name="ident")
            make_identity(nc, ident)
            wq_sb = consts.tile([128, 512], f16, name="wq_sb")
            wk_sb = consts.tile([128, 512], f16, name="wk_sb")
            wv_sb = consts.tile([128, 1024], f16, name="wv_sb")
            bq_sb = consts.tile([128, 1], f32, name="bq_sb")
            bk_sb = consts.tile([128, 1], f32, name="bk_sb")
            bvb_sb = consts.tile([128, 256], f32, name="bvb_sb")
            xt = [persist.tile([128, S], f16, name=f"xt{i}") for i in range(4)]
            qt = persist.tile([128, S], f16, name="qt")
            kt = persist.tile([128, S], f16, name="kt")
            # v1[:, kc*256 + h*128 : +128] = [V_h(64) | ones | 63 zero pad]
            # (full 128-col weight tiles keep the EV LDWEIGHTS on the FWL path)
            v1 = persist.tile([128, n_kc * 256], f16, name="v1")

            # first xT block ahead of the weights: it gates the first
            # K-projection chunk
            for dc in range(4):
                for hh in range(2):
                    cs = slice(hh * 256, (hh + 1) * 256)
                    nc.sync.dma_start(xt[dc][:, cs],
                                      xT[dc * 128:(dc + 1) * 128, cs])
            for dc in range(4):
                r = slice(dc * 128, (dc + 1) * 128)
                nc.sync.dma_start(wq_sb[:, dc * 128:(dc + 1) * 128], wqT[r, :])
                nc.sync.dma_start(wk_sb[:, dc * 128:(dc + 1) * 128], wkT[r, :])
                nc.sync.dma_start(wv_sb[:, dc * 256:(dc + 1) * 256], wvT[r, :])
            nc.sync.dma_start(bq_sb[:], bq[:, :])
            nc.sync.dma_start(bk_sb[:], bk[:, :])
            nc.sync.dma_start(bvb_sb[:], bvb[:, :])

            # ---------------- shared PSUM ring (3 x [128,1024] = 6 banks) --
            # Projections and attention score tiles rotate through the same
            # three tags via a single global counter, so the ring stays
            # round-robin even while qc0's attention interleaves with the
            # tail of the projections.
            st_ctr = [0]

            def st_alloc(name):
                t = stp.tile([128, 1024], f32, name=name,
                             tag=f"st{st_ctr[0] % 3}")
                st_ctr[0] += 1
                return t

            # ---------------- projection emitters ----------------
            def emit_k_chunk(cs):
                pk = st_alloc("pk")
                for dc in range(4):
                    nc.tensor.matmul(
                        pk[:, 0:512],
                        lhsT=wk_sb[:, dc * 128:(dc + 1) * 128],
                        rhs=xt[dc][:, cs],
                        start=(dc == 0),
                        stop=(dc == 3),
                    )
                nc.scalar.activation(kt[:, cs], pk[:, 0:512], AF.Identity,
                                     bias=bk_sb[:])

            def emit_q_chunk(cs):
                pq = st_alloc("pq")
                for dc in range(4):
                    nc.tensor.matmul(
                        pq[:, 0:512],
                        lhsT=wq_sb[:, dc * 128:(dc + 1) * 128],
                        rhs=xt[dc][:, cs],
                        start=(dc == 0),
                        stop=(dc == 3),
                    )
                nc.scalar.activation(qt[:, cs], pq[:, 0:512], AF.Identity,
                                     bias=bq_sb[:])

            def emit_v_ktile(kc):
                ks = slice(kc * 128, (kc + 1) * 128)
                pv = st_alloc("pv")
                for dc in range(4):
                    nc.tensor.matmul(
                        pv[:, 0:256],
                        lhsT=xt[dc][:, ks],
                        rhs=wv_sb[:, dc * 256:(dc + 1) * 256],
                        start=(dc == 0),
                        stop=(dc == 3),
                    )
                nc.vector.tensor_add(
                    v1[:, kc * 256:(kc + 1) * 256], pv[:, 0:256], bvb_sb[:]
                )

            # ---------------- attention emitters ----------------
            pend_tail = [None]   # deferred kc=31 EV + epilogue of prev qc
            cur = {}             # per-qc state: po tiles, et ring

            def emit_ev(qc, kc, et):
                for h in range(2):
                    nc.tensor.matmul(
                        cur["po"][h][:],
                        lhsT=v1[:, kc * 256 + h * 128: kc * 256 + (h + 1) * 128],
                        rhs=et[:, h * 512:(h + 1) * 512],
                        start=(kc == 0),
                        stop=(kc == n_kc - 1),
                    )

            def emit_tail(po, qc, et31):
                # kc=31 EV and the PSUM->SBUF copy interleaved per head, so
                # the next qc's first EV (which reuses these PSUM banks) is
                # unblocked as early as possible.
                last = qc == n_qc - 1
                res = [
                    outp.tile([128, 128], f32, name=f"res{t}", tag=f"res{t}")
                    for t in range(4)
                ]
                for h in range(2):
                    nc.tensor.matmul(
                        po[h][:],
                        lhsT=v1[:, (n_kc - 1) * 256 + h * 128:
                                (n_kc - 1) * 256 + (h + 1) * 128],
                        rhs=et31[:, h * 512:(h + 1) * 512],
                        start=False,
                        stop=True,
                    )
                    ot = outp.tile([128, 512], f16, name="ot", tag=f"ot{h}")
                    nc.scalar.activation(ot[0:80, :], po[h][0:80, :], AF.Identity)
                    for t in range(4):
                        if last:
                            # PE transpose (xbar DMA transposes would
                            # serialize on the exposed kernel tail)
                            pt = stp.tile([128, 65], f16, name="pt",
                                          tag=f"st{st_ctr[0] % 3}")
                            st_ctr[0] += 1
                            nc.tensor.transpose(
                                pt[:],
                                ot[0:65, t * 128:(t + 1) * 128],
                                ident[0:65, 0:65],
                            )
                            src = pt
                        else:
                            tp = outp.tile([128, 80], f16, name="tp", tag="tp")
                            nc.sync.dma_start_transpose(
                                tp[:], ot[0:80, t * 128:(t + 1) * 128]
                            )
                            src = tp
                        rcp = outp.tile([128, 1], f32, name="rcp", tag="rcp")
                        nc.vector.reciprocal(rcp[:], src[:, 64:65])
                        nc.vector.tensor_scalar_mul(
                            res[t][:, h * 64:(h + 1) * 64], src[:, 0:64], rcp[:]
                        )
                for t in range(4):
                    nc.sync.dma_start(
                        out[qc * 512 + t * 128: qc * 512 + (t + 1) * 128, :],
                        res[t][:],
                    )

            def attn_batches(qc, kc_lo, kc_hi):
                qs = slice(qc * 512, (qc + 1) * 512)
                for kc in range(kc_lo, kc_hi):
                    if kc == 0:
                        cur["et"] = {}
                    st = st_alloc("st")
                    for h in range(2):
                        hp = slice(h * 64, (h + 1) * 64)
                        nc.tensor.matmul(
                            st[:, h * 512:(h + 1) * 512],
                            lhsT=kt[hp, kc * 128:(kc + 1) * 128],
                            rhs=qt[hp, qs],
                            start=True,
                            stop=True,
                        )
                    if kc == 0:
                        # previous qc's tail EV + epilogue, deferred past
                        # this qc's first scores so the PE stays fed
                        if pend_tail[0] is not None:
                            pend_tail[0]()
                            pend_tail[0] = None
                    et = etp.tile([128, 1024], f16, name="et", tag="et")
                    if _on_act(kc):
                        nc.scalar.activation(et[:], st[:], AF.Exp, scale=0.125)
                    else:
                        nc.vector.tensor_scalar(
                            et[:].bitcast(i16), st[:],
                            SCHR_ALPHA, SCHR_BETA, ALU.mult, ALU.add,
                        )
                    cur["et"][kc] = et
                    if kc == 3:
                        # allocate this qc's O^T accumulators only now: the
                        # previous qc's epilogue reads (same PSUM banks,
                        # bufs=1) are already emitted above, so the WAR
                        # dependency is tracked.
                        cur["po"] = [
                            pop.tile([128, 512], f32, name=f"po{h}",
                                     tag=f"po{h}")
                            for h in range(2)
                        ]
                    if kc >= 3:
                        emit_ev(qc, kc - 3, cur["et"].pop(kc - 3))
                if kc_hi == n_kc:
                    emit_ev(qc, n_kc - 3, cur["et"].pop(n_kc - 3))
                    emit_ev(qc, n_kc - 2, cur["et"].pop(n_kc - 2))
                    et31 = cur["et"].pop(n_kc - 1)
                    po = cur["po"]
                    pend_tail[0] = (
                        lambda po=po, qc=qc, et=et31: emit_tail(po, qc, et)
                    )

            # ---------------- emission: blocks + pipelined qc0 ----------
            blocks = [(0, 512), (512, 512), (1024, 1024), (2048, 1024),
                      (3072, 1024)]
            q_sched = {0: [0], 1: [1], 2: [2, 3], 3: [4, 5], 4: [6, 7]}
            att_sched = {1: (0, 8), 2: (8, 16), 3: (16, 24), 4: (24, 32)}
            for bi, (boff, blk) in enumerate(blocks):
                for dc in range(4):
                    if bi == 0:
                        continue  # block0 DMA'd ahead of the weights above
                    cs = slice(boff, boff + blk)
                    nc.sync.dma_start(xt[dc][:, cs],
                                      xT[dc * 128:(dc + 1) * 128, cs])
                for half in range(blk // 512):
                    hs = slice(boff + half * 512, boff + (half + 1) * 512)
                    emit_k_chunk(hs)
                for kci in range(boff // 128, (boff + blk) // 128):
                    emit_v_ktile(kci)
                for qi in q_sched[bi]:
                    emit_q_chunk(slice(qi * 512, (qi + 1) * 512))
                if bi in att_sched:
                    attn_batches(0, *att_sched[bi])

            for qc in range(1, n_qc):
                attn_batches(qc, 0, n_kc)
            pend_tail[0]()
    return nc


def _shard_inputs(x, Wq, bq, Wk, bk, Wv, bv):
    """Build the 8 per-core input maps from full inputs."""
    x = np.asarray(x, dtype=np.float32)
    in_maps = []
    for c in range(N_CORES):
        b, pair = c // 4, c % 4
        rows = slice(pair * 128, (pair + 1) * 128)
        wq_s = np.asarray(Wq)[rows, :].astype(np.float32)
        wk_s = np.asarray(Wk)[rows, :].astype(np.float32)
        wv_s = np.asarray(Wv)[rows, :].astype(np.float32)
        bq_s = np.asarray(bq)[rows].astype(np.float32)
        bk_s = np.asarray(bk)[rows].astype(np.float32)
        bv_s = np.asarray(bv)[rows].astype(np.float32)

        wvT = np.zeros((D_MODEL, 256), np.float32)
        wvT[:, 0:64] = wv_s[0:64].T
        wvT[:, 128:192] = wv_s[64:128].T
        bvb = np.zeros((128, 256), np.float32)
        bvb[:, 0:64] = bv_s[0:64]
        bvb[:, 64] = 1.0
        bvb[:, 128:192] = bv_s[64:128]
        bvb[:, 192] = 1.0

        in_maps.append({
            "xT": np.ascontiguousarray(x[c // 4].T).astype(np.float16),
            "wqT": np.ascontiguousarray(wq_s.T).astype(np.float16),
            "wkT": np.ascontiguousarray(wk_s.T).astype(np.float16),
            "wvT": wvT.astype(np.float16),
            "bq": bq_s.reshape(128, 1).copy(),
            "bk": bk_s.reshape(128, 1).copy(),
            "bvb": bvb,
        })
    return in_maps


def _gather(results):
    B, S, D = 2, S_FULL, D_MODEL
    out = np.empty((B, S, D), np.float32)
    for c in range(N_CORES):
        b, pair = c // 4, c % 4
        out[b, :, pair * 128:(pair + 1) * 128] = results[c]["out"]
    return out


def _install_profile_hook():
    """Provide antenv.axon_hooks (missing in this image) so that
    run_bass_kernel_spmd(trace=True) can capture NTFF profiles, using the
    same ctypes path trn_boot.py would have registered."""
    import sys, types, ctypes, contextlib

    if "antenv.axon_hooks" in sys.modules:
        return
    so_path = "/opt/axon/libaxon_pjrt.so"
    mod = types.ModuleType("antenv.axon_hooks")
    state = {"hook": None}
    mod.set_axon_ntff_profile_hook = lambda h: state.__setitem__("hook", h)
    mod.get_axon_ntff_profile_hook = lambda: state["hook"]
    sys.modules["antenv.axon_hooks"] = mod
    try:
        lib = ctypes.CDLL(so_path)
        if not hasattr(lib, "axon_start_nrt_profile"):
            return
        lib.axon_start_nrt_profile.argtypes = [
            ctypes.POINTER(ctypes.c_int64), ctypes.c_size_t]
        lib.axon_start_nrt_profile.restype = ctypes.c_int64
        lib.axon_stop_nrt_profile.argtypes = [ctypes.c_char_p]
        lib.axon_stop_nrt_profile.restype = ctypes.c_int64

        @contextlib.contextmanager
        def _hook(output_dir, device_ids):
            import jax
            jax.devices()
            if device_ids:
                ids = (ctypes.c_int64 * len(device_ids))(*device_ids)
                rc = lib.axon_start_nrt_profile(ids, len(device_ids))
            else:
                rc = lib.axon_start_nrt_profile(None, 0)
            if rc != 0:
                raise RuntimeError(f"axon_start_nrt_profile rc={rc}")
            try:
                yield
            finally:
                n = lib.axon_stop_nrt_profile(str(output_dir).encode())
                print(f"profile: {n} file(s) written to {output_dir}")

        state["hook"] = _hook
    except OSError:
        pass


def kernel(x, Wq, bq, Wk, bk, Wv, bv, trace=False):
    from concourse.bass_utils import run_bass_kernel_spmd

    if trace:
        _install_profile_hook()
    if "nc" not in _cached:
        nc = build_nc(S_FULL)
        nc.finalize()
        _cached["nc"] = nc
    nc = _cached["nc"]
    in_maps = _shard_inputs(x, Wq, bq, Wk, bk, Wv, bv)
    r = run_bass_kernel_spmd(nc, in_maps, list(range(N_CORES)), trace=trace)
    _cached["last_results"] = r
    return _gather(r.results)
